# revision 65
# baseline (speedup 1.0000x reference)
"""GAT-style bipartite graph attention layer (nn_BiGraphContrastLayer) on 8 trn2 cores.

Strategy (dst-sharded SPMD, one shared program):
  - Each core owns 1250 dst nodes (10 tiles of 128).  Phase 1 computes
    z/el/er = x @ [W | W@Al | W@Ar] for all 10000 src nodes (79 tiles,
    written as a DRAM row table zel_tab[s] = [z(512) | el(8)], 640-elem
    stride) and for its 10 dst tiles (kept in SBUF).
  - Real edges (no self loops), grouped by dst tile and sorted by src, are
    fetched per edge via SWDGE dma_gather.  Edges with consecutive src ids
    (s, s+1) are paired into one 2560B descriptor (gather elem = 2 rows via
    an overlapping-window source AP); the rest use 1280B single-row
    descriptors.  This cuts Q7 descriptor-emission time, the phase-2 serial
    bottleneck.
  - v = exp(leaky_relu(el_src + er_dst)) per edge/head (er broadcast to edges
    via one-hot selD matmuls on the PE); msg = v * z_src (DVE); per-dst-tile
    segment sums via one-hot selT matmuls accumulating in PSUM.  Self-loop
    contribution (v_self, v_self*z_dst) is added from the SBUF-resident dst
    tiles.  Final: (po + v_self*z_dst)/(ps + v_self) + bias.
  No inter-core communication; host concatenates the 8 dst slices.
"""
import os

import numpy as np
import ml_dtypes

import bass_rust
import concourse.bacc as bacc
import concourse.bass as bass
import concourse.mybir as mybir
import concourse.tile as tile

BF = ml_dtypes.bfloat16
F32 = np.float32

NS, ND, E, DIN, H, DH = 10000, 10000, 320000, 512, 8, 64
NEG = 0.2
NCORES = 8
DPC = ND // NCORES            # 1250 dst nodes per core
NDSTT = 10                    # dst tiles per core
SRCPAD = 10112                # src rows padded to 79 tiles of 128
NSRCT = SRCPAD // 128         # 79
NXT = NSRCT + NDSTT           # 89 xT tiles; 0..78 src, 79..88 dst
ROW = 640                     # zel row stride elems (bf16); 520 used
PANEL = 1024                  # phase-1 node panel (4 subtiles of 128)
NXTP = 96                    # xT tiles padded to a PANEL multiple
ZG_BUFS = 2


# ----------------------------------------------------------------- host prep
def _wrap_idx(idx, k):
    """dma_gather index layout: idx i -> [i % 16, i // 16], replicated 8x.
    idx shorter than k is padded with 0 (row 0 gathered, masked by sel)."""
    full = np.zeros(k, np.int64)
    full[:len(idx)] = idx
    w = np.zeros((16, k // 16), np.int16)
    w[np.arange(k) % 16, np.arange(k) // 16] = full
    return np.tile(w, (8, 1))


def _pack_tile(srcs, dsts):
    """Greedy pair edges with consecutive src ids.

    Returns (idx2, dst2a, dst2b, idx1, dst1): R2 descriptors cover rows
    (s, s+1) for edge pairs; R1 descriptors cover single rows."""
    order = np.argsort(srcs, kind="stable")
    s, d = srcs[order], dsts[order]
    n = len(s)
    idx2, d2a, d2b, idx1, d1 = [], [], [], [], []
    i = 0
    while i < n:
        if i + 1 < n and s[i + 1] == s[i] + 1:
            idx2.append(s[i]); d2a.append(d[i]); d2b.append(d[i + 1])
            i += 2
        else:
            idx1.append(s[i]); d1.append(d[i])
            i += 1
    return (np.array(idx2, np.int64), np.array(d2a, np.int64),
            np.array(d2b, np.int64), np.array(idx1, np.int64),
            np.array(d1, np.int64))


def _host_prep(x_src, x_dst, edge_src, edge_dst, W, attn_l, attn_r, bias):
    Al = np.zeros((DIN, H), F32)
    Ar = np.zeros((DIN, H), F32)
    for h in range(H):
        Al[h * DH:(h + 1) * DH, h] = attn_l[h]
        Ar[h * DH:(h + 1) * DH, h] = attn_r[h]
    Wext = np.concatenate([W, W @ Al, W @ Ar], 1).astype(BF)  # [512, 528]
    bias_rep = np.tile(bias[None, :].astype(F32), (128, 1))   # [128, 512]

    edge_src = edge_src.astype(np.int64)
    edge_dst = edge_dst.astype(np.int64)

    packs = [[None] * NDSTT for _ in range(NCORES)]
    k2s = [0] * NDSTT
    k1s = [0] * NDSTT
    for c in range(NCORES):
        d0 = c * DPC
        m = (edge_dst >= d0) & (edge_dst < d0 + DPC)
        es, ed = edge_src[m], edge_dst[m] - d0
        for t in range(NDSTT):
            sel = (ed >= t * 128) & (ed < (t + 1) * 128)
            pk = _pack_tile(es[sel], ed[sel] - t * 128)
            packs[c][t] = pk
            k2s[t] = max(k2s[t], len(pk[0]))
            k1s[t] = max(k1s[t], len(pk[3]))
    k2s = [((k + 127) // 128) * 128 for k in k2s]
    k1s = [((k + 127) // 128) * 128 for k in k1s]
    nchs = [2 * ((k2 + 127) // 128) + (k1 + 127) // 128
            for k2, k1 in zip(k2s, k1s)]
    o2 = np.cumsum([0] + k2s).tolist()     # zidx2 offsets (idx units)
    o1 = np.cumsum([0] + k1s).tolist()
    osel = np.cumsum([0] + [n * 128 for n in nchs]).tolist()

    per_core = []
    for c in range(NCORES):
        zidx2 = np.zeros((128, o2[-1] // 16), np.int16)
        zidx1 = np.zeros((128, o1[-1] // 16), np.int16)
        selT = np.zeros((128, osel[-1]), BF)
        selD = np.zeros((128, osel[-1]), BF)
        for t in range(NDSTT):
            k2, k1, nch = k2s[t], k1s[t], nchs[t]
            c2 = (k2 + 127) // 128
            idx2, d2a, d2b, idx1, d1 = packs[c][t]
            zidx2[:, o2[t] // 16:o2[t + 1] // 16] = _wrap_idx(idx2, k2)
            zidx1[:, o1[t] // 16:o1[t + 1] // 16] = _wrap_idx(idx1, k1)
            # edge -> (partition, chunk) slots
            sm = np.zeros((nch, 128, 128), F32)   # [chunk, edge part, dst]
            j2 = np.arange(len(idx2))
            sm[2 * (j2 // 128), j2 % 128, d2a] = 1.0
            sm[2 * (j2 // 128) + 1, j2 % 128, d2b] = 1.0
            j1 = np.arange(len(idx1))
            sm[2 * c2 + j1 // 128, j1 % 128, d1] = 1.0
            base = osel[t]
            for ch in range(nch):
                selT[:, base + ch * 128:base + (ch + 1) * 128] = \
                    sm[ch].astype(BF)
                selD[:, base + ch * 128:base + (ch + 1) * 128] = \
                    sm[ch].T.astype(BF)

        # xT: cols 0..9999 src nodes, cols 10112.. own dst slice
        xc = np.zeros((NXTP * 128, DIN), F32)
        xc[:NS] = x_src
        xc[SRCPAD:SRCPAD + DPC] = x_dst[c * DPC:(c + 1) * DPC]
        xT = np.ascontiguousarray(xc.T).astype(BF)            # [512, 11776]
        per_core.append(dict(xT=xT, selT=selT, selD=selD,
                             zidx2=zidx2, zidx1=zidx1))

    shared = dict(Wext=Wext, bias_rep=bias_rep)
    return shared, per_core, k2s, k1s


# ------------------------------------------------------------- bass program
def _build_nc(k2s, k1s):
    nchs = [2 * ((k2 + 127) // 128) + (k1 + 127) // 128
            for k2, k1 in zip(k2s, k1s)]
    o2 = np.cumsum([0] + k2s).tolist()
    o1 = np.cumsum([0] + k1s).tolist()
    osel = np.cumsum([0] + [n * 128 for n in nchs]).tolist()
    nch_max = max(nchs)
    nc = bacc.Bacc("TRN2", target_bir_lowering=False, debug=False)
    dt = mybir.dt

    xT_d = nc.dram_tensor("xT", [DIN, NXTP * 128], dt.bfloat16,
                          kind="ExternalInput")
    W_d = nc.dram_tensor("Wext", [DIN, 528], dt.bfloat16, kind="ExternalInput")
    bias_d = nc.dram_tensor("bias_rep", [128, 512], dt.float32,
                            kind="ExternalInput")
    selT_d = nc.dram_tensor("selT", [128, osel[-1]], dt.bfloat16,
                            kind="ExternalInput")
    selD_d = nc.dram_tensor("selD", [128, osel[-1]], dt.bfloat16,
                            kind="ExternalInput")
    zidx2_d = nc.dram_tensor("zidx2", [128, o2[-1] // 16], dt.int16,
                             kind="ExternalInput")
    zidx1_d = nc.dram_tensor("zidx1", [128, o1[-1] // 16], dt.int16,
                             kind="ExternalInput")
    out_d = nc.dram_tensor("out", [NDSTT * 128, 512], dt.float32,
                           kind="ExternalOutput")
    zel_d = nc.dram_tensor("zel_tab", [SRCPAD, ROW], dt.bfloat16)

    with tile.TileContext(nc) as tc:
        with tc.tile_pool(name="const", bufs=1) as cpool:
            wsb = cpool.tile([128, 4 * 528], dt.bfloat16)
            for k in range(4):
                nc.sync.dma_start(wsb[:, k * 528:(k + 1) * 528],
                                  W_d[k * 128:(k + 1) * 128, :])
            bias_sb = cpool.tile([128, 512], dt.float32)
            nc.sync.dma_start(bias_sb[:], bias_d[:])
            zidx2_sb = cpool.tile([128, o2[-1] // 16], dt.int16)
            nc.sync.dma_start(zidx2_sb[:], zidx2_d[:])
            zidx1_sb = cpool.tile([128, o1[-1] // 16], dt.int16)
            nc.sync.dma_start(zidx1_sb[:], zidx1_d[:])
            zdst = cpool.tile([128, NDSTT, 528], dt.bfloat16)
            vself = cpool.tile([128, NDSTT, 8], dt.float32)

            with tc.tile_pool(name="zg", bufs=ZG_BUFS) as zgpool:
                zgt = [zgpool.tile([128, nch_max, ROW], dt.bfloat16,
                                   tag="zg", name=f"zg{i}")
                       for i in range(NDSTT)]

                # ---- phase 1: z/el for src tiles (DRAM zel_tab) + z/el/er
                # for dst tiles (SBUF).  Tiles 0..78 src, 79..88 dst.
                ph1 = tc.tile_pool(name="xp", bufs=2)
                xpool = ph1.__enter__()

                # zel write buffers: full 640-elem rows with the 520:640 tail
                # zeroed once, so zel_tab holds no garbage (gathers read whole
                # rows).
                zel_bufs = [cpool.tile([128, ROW], dt.bfloat16,
                                       name=f"zelbuf{i}") for i in range(3)]
                for zb in zel_bufs:
                    nc.vector.memset(zb[:, 520:ROW], 0)
                ph1c = tc.tile_pool(name="p1", bufs=3, space="PSUM")
                p1pool = ph1c.__enter__()
                ph1d = tc.tile_pool(name="p1b", bufs=3, space="PSUM")
                p1bpool = ph1d.__enter__()
                for p in range(NXTP * 128 // PANEL):
                    xp = xpool.tile([128, 4 * PANEL], dt.bfloat16)
                    for k in range(4):
                        nc.sync.dma_start(
                            xp[:, k * PANEL:(k + 1) * PANEL],
                            xT_d[k * 128:(k + 1) * 128,
                                 p * PANEL:(p + 1) * PANEL])
                    for m in range(PANEL // 128):
                        gt = p * (PANEL // 128) + m   # global tile index
                        if gt >= NXT:
                            continue
                        zps = p1pool.tile([128, 512], dt.float32, space="PSUM")
                        lps = p1bpool.tile([128, 16], dt.float32, space="PSUM")
                        for k in range(4):
                            lhsT = xp[:, k * PANEL + m * 128:
                                      k * PANEL + (m + 1) * 128]
                            nc.tensor.matmul(zps[:], lhsT,
                                             wsb[:, k * 528:k * 528 + 512],
                                             start=(k == 0), stop=(k == 3))
                        for k in range(4):
                            lhsT = xp[:, k * PANEL + m * 128:
                                      k * PANEL + (m + 1) * 128]
                            nc.tensor.matmul(lps[:], lhsT,
                                             wsb[:, k * 528 + 512:(k + 1) * 528],
                                             start=(k == 0), stop=(k == 3))
                        if gt >= NSRCT:
                            # dst tile: keep z|el|er in SBUF
                            dst_t = gt - NSRCT
                            if gt % 2 == 0:
                                nc.vector.tensor_copy(zdst[:, dst_t, 0:512],
                                                      zps[:])
                                nc.scalar.activation(
                                    zdst[:, dst_t, 512:528], lps[:],
                                    mybir.ActivationFunctionType.Copy)
                            else:
                                nc.scalar.activation(
                                    zdst[:, dst_t, 0:512], zps[:],
                                    mybir.ActivationFunctionType.Copy)
                                nc.vector.tensor_copy(zdst[:, dst_t, 512:528],
                                                      lps[:])
                        else:
                            st = gt                   # src tile index
                            zel_sb = zel_bufs[st % 3]
                            if gt % 2 == 0:
                                nc.vector.tensor_copy(zel_sb[:, 0:512], zps[:])
                                nc.scalar.activation(
                                    zel_sb[:, 512:520], lps[:, 0:8],
                                    mybir.ActivationFunctionType.Copy)
                            else:
                                nc.scalar.activation(
                                    zel_sb[:, 0:512], zps[:],
                                    mybir.ActivationFunctionType.Copy)
                                nc.vector.tensor_copy(zel_sb[:, 512:520],
                                                      lps[:, 0:8])
                            row0 = st * 128
                            nc.sync.dma_start(zel_d[row0:row0 + 128, :],
                                              zel_sb[:])
                ph1d.__exit__(None, None, None)
                ph1c.__exit__(None, None, None)
                ph1.__exit__(None, None, None)

                # vself[d, t, h] = exp(lrelu(el + er)) for the dst self loops
                vtmp = cpool.tile([128, NDSTT, 8], dt.float32)
                nc.vector.tensor_tensor(vtmp[:], zdst[:, :, 512:520],
                                        zdst[:, :, 520:528],
                                        op=mybir.AluOpType.add)
                nc.vector.scalar_tensor_tensor(
                    vtmp[:], vtmp[:], NEG, vtmp[:],
                    op0=mybir.AluOpType.mult, op1=mybir.AluOpType.max)
                nc.scalar.activation(vself[:], vtmp[:],
                                     mybir.ActivationFunctionType.Exp)

                # R2 gathers read 2 consecutive table rows per descriptor:
                # overlapping-window source AP [[640, SRCPAD-1], [1, 1280]]
                zel2_ap = zel_d[:].copy()
                zel2_ap.ap = bass_rust.VecI64Pair(
                    [[ROW, SRCPAD - 1], [1, 2 * ROW]])

                # ---- phase 2: per dst tile gather + attention + aggregation
                with (
                    tc.tile_pool(name="sel", bufs=2) as selpool,
                    tc.tile_pool(name="sc", bufs=3) as scpool,
                    tc.tile_pool(name="vx", bufs=2) as vxpool,
                    tc.tile_pool(name="eo", bufs=2) as eopool,
                    tc.tile_pool(name="p2", bufs=2, space="PSUM") as p2pool,
                    tc.tile_pool(name="p2b", bufs=2, space="PSUM") as p2bpool,
                    tc.tile_pool(name="p2c", bufs=2, space="PSUM") as p2cpool,
                ):
                    def emit_seld_peer(t):
                        # seld load + er-broadcast matmuls for tile t.  Called
                        # one tile ahead so pe_er(t) is not queued behind the
                        # full po/ps chain of tile t-1 on the PE.
                        ncht = nchs[t]
                        seld = selpool.tile([128, nch_max * 128],
                                            dt.bfloat16, tag="seld",
                                            name=f"seld{t}")
                        nc.sync.dma_start(
                            seld[:, 0:ncht * 128],
                            selD_d[:, osel[t]:osel[t + 1]])
                        pe_er = p2cpool.tile([128, nch_max, 8], dt.float32,
                                             space="PSUM", tag="peer", name=f"peer{t}")
                        for ch in range(ncht):
                            nc.tensor.matmul(pe_er[:, ch, :],
                                             seld[:, ch * 128:(ch + 1) * 128],
                                             zdst[:, t, 520:528],
                                             start=True, stop=True,
                                             skip_group_check=True)
                        return pe_er

                    pe_ers = {0: emit_seld_peer(0)}
                    for t in range(NDSTT):
                        k2, k1, nch = k2s[t], k1s[t], nchs[t]
                        c2 = (k2 + 127) // 128
                        last_tile = t == NDSTT - 1
                        sel = selpool.tile([128, nch_max * 128], dt.bfloat16,
                                           tag="sel")
                        nc.sync.dma_start(
                            sel[:, 0:nch * 128],
                            selT_d[:, osel[t]:osel[t + 1]])

                        zg = zgt[t]

                        def gather_r2():
                            zg2 = zg[:, 0:2 * c2, :].rearrange(
                                "p (a b) c -> p a (b c)", b=2)
                            nc.gpsimd.dma_gather(
                                zg2, zel2_ap,
                                zidx2_sb[:, o2[t] // 16:o2[t + 1] // 16],
                                num_idxs=k2, num_idxs_reg=k2,
                                elem_size=2 * ROW, elem_step=ROW,
                                single_packet=False)

                        def gather_r1():
                            nc.gpsimd.dma_gather(
                                zg[:, 2 * c2:nch, :], zel_d[:],
                                zidx1_sb[:, o1[t] // 16:o1[t + 1] // 16],
                                num_idxs=k1, num_idxs_reg=k1, elem_size=ROW,
                                single_packet=False)

                        # For the last tile, gather R1 first so the final
                        # post-gather chain is the short R2 pass.
                        regions = ((0, 2 * c2), (2 * c2, nch))
                        if last_tile:
                            gather_r1()
                            gather_r2()
                            regions = ((2 * c2, nch), (0, 2 * c2))
                        else:
                            gather_r2()
                            gather_r1()

                        pe_er = pe_ers.pop(t)
                        first_ch = regions[0][0]
                        last_ch = regions[-1][1] - 1

                        lt = scpool.tile([128, nch_max, 8], dt.float32,
                                         tag="lt")
                        vb = scpool.tile([128, nch_max, 8], dt.bfloat16,
                                         tag="vb")
                        po = p2pool.tile([128, 512], dt.float32, space="PSUM")
                        ps = p2bpool.tile([128, 8], dt.float32, space="PSUM")
                        GRP = 8
                        for ri, (r0, r1) in enumerate(regions):
                            nc.vector.tensor_tensor(
                                lt[:, r0:r1, :], zg[:, r0:r1, 512:520],
                                pe_er[:, r0:r1, :], op=mybir.AluOpType.add)
                            nc.vector.scalar_tensor_tensor(
                                lt[:, r0:r1, :], lt[:, r0:r1, :], NEG,
                                lt[:, r0:r1, :],
                                op0=mybir.AluOpType.mult,
                                op1=mybir.AluOpType.max)
                            nc.scalar.activation(
                                vb[:, r0:r1, :], lt[:, r0:r1, :],
                                mybir.ActivationFunctionType.Exp)
                            # msg = v * z (in place) interleaved with the
                            # segment sums per chunk group.  v is expanded to
                            # full 512 width on the scalar engine so the DVE
                            # multiply runs both operands step-1 (2x bf16).
                            for g0 in range(r0, r1, GRP):
                                g1 = min(g0 + GRP, r1)
                                ng = g1 - g0
                                vbig = vxpool.tile([128, GRP, 512],
                                                   dt.bfloat16, tag="vbig")
                                v4 = vbig[:, 0:ng, :].rearrange(
                                    "p c (h d) -> p c h d", d=DH)
                                nc.scalar.activation(
                                    v4,
                                    vb[:, g0:g1, :].to_broadcast(
                                        [128, ng, 8, DH]),
                                    mybir.ActivationFunctionType.Copy)
                                nc.vector.tensor_tensor(
                                    zg[:, g0:g1, 0:512],
                                    zg[:, g0:g1, 0:512],
                                    vbig[:, 0:ng, :],
                                    op=mybir.AluOpType.mult)
                                for ch in range(g0, g1):
                                    sl = sel[:, ch * 128:(ch + 1) * 128]
                                    nc.tensor.matmul(po[:], sl,
                                                     zg[:, ch, 0:512],
                                                     start=(ch == first_ch),
                                                     stop=(ch == last_ch))
                                    nc.tensor.matmul(ps[:], sl, vb[:, ch, :],
                                                     start=(ch == first_ch),
                                                     stop=(ch == last_ch))
                            # hoist next tile's seld load + er matmuls ahead
                            # of this tile's second region pass on the PE
                            if ri == 0 and t + 1 < NDSTT:
                                pe_ers[t + 1] = emit_seld_peer(t + 1)

                        # out = (po + vself*z_dst) / (ps + vself) + bias
                        ssb = scpool.tile([128, 8], dt.float32, tag="ssb")
                        nc.vector.tensor_tensor(ssb[:], ps[:], vself[:, t, :],
                                                op=mybir.AluOpType.add)
                        nc.vector.reciprocal(ssb[:], ssb[:])
                        msf = scpool.tile([128, 512], dt.float32, tag="msf")
                        m4 = msf[:].rearrange("p (h d) -> p h d", d=DH)
                        nc.vector.tensor_tensor(
                            m4, zdst[:, t, 0:512].rearrange(
                                "p (h d) -> p h d", d=DH),
                            vself[:, t, :].to_broadcast([128, 8, DH]),
                            op=mybir.AluOpType.mult)
                        osb = eopool.tile([128, 512], dt.float32)
                        nc.vector.tensor_tensor(osb[:], po[:], msf[:],
                                                op=mybir.AluOpType.add)
                        o4 = osb[:].rearrange("p (h d) -> p h d", d=DH)
                        nc.vector.tensor_tensor(
                            o4, o4, ssb[:].to_broadcast([128, 8, DH]),
                            op=mybir.AluOpType.mult)
                        nc.vector.tensor_tensor(osb[:], osb[:], bias_sb[:],
                                                op=mybir.AluOpType.add)
                        nc.sync.dma_start(out_d[t * 128:(t + 1) * 128, :],
                                          osb[:])
    nc.compile()
    return nc


# ------------------------------------------------------------------- driver
def kernel(x_src, x_dst, edge_src, edge_dst, W, attn_l, attn_r, bias):
    shared, per_core, k2s, k1s = _host_prep(
        np.asarray(x_src), np.asarray(x_dst), np.asarray(edge_src),
        np.asarray(edge_dst), np.asarray(W), np.asarray(attn_l),
        np.asarray(attn_r), np.asarray(bias))

    nc = _build_nc(k2s, k1s)

    in_maps = []
    for c in range(NCORES):
        in_maps.append({"xT": per_core[c]["xT"], "Wext": shared["Wext"],
                        "bias_rep": shared["bias_rep"],
                        "selT": per_core[c]["selT"],
                        "selD": per_core[c]["selD"],
                        "zidx2": per_core[c]["zidx2"],
                        "zidx1": per_core[c]["zidx1"]})

    if os.environ.get("KERNEL_SIM"):
        from concourse.bass_interp import CoreSim
        sim = CoreSim(nc, trace=False)
        for name, arr in in_maps[int(os.environ.get("KERNEL_SIM_CORE", "0"))].items():
            sim.tensor(name)[:] = arr
        sim.simulate()
        out = np.array(sim.tensor("out"))
        return np.concatenate([out[:DPC]] * NCORES, 0)  # core-0 slice only

    from concourse.bass_utils import run_bass_kernel_spmd
    res = run_bass_kernel_spmd(nc, in_maps, core_ids=list(range(NCORES)),
                               trace=bool(os.environ.get("KERNEL_TRACE")))
    global LAST_RESULTS
    LAST_RESULTS = res
    return np.concatenate([r["out"][:DPC] for r in res.results], 0)


LAST_RESULTS = None


# revision 66
# speedup vs baseline: 1.0350x; 1.0350x over previous
"""GAT-style bipartite graph attention layer (nn_BiGraphContrastLayer) on 8 trn2 cores.

Strategy (dst-sharded SPMD, one shared program):
  - Each core owns 1250 dst nodes (10 tiles of 128).  Phase 1 computes
    z/el/er = x @ [W | W@Al | W@Ar] for all 10000 src nodes (79 tiles,
    written as a DRAM row table zel_tab[s] = [z(512) | el(8)], 640-elem
    stride) and for its 10 dst tiles (kept in SBUF).
  - Real edges (no self loops), grouped by dst tile and sorted by src, are
    fetched per edge via SWDGE dma_gather.  Edges with consecutive src ids
    (s, s+1) are paired into one 2560B descriptor (gather elem = 2 rows via
    an overlapping-window source AP); the rest use 1280B single-row
    descriptors.  This cuts Q7 descriptor-emission time, the phase-2 serial
    bottleneck.
  - v = exp(leaky_relu(el_src + er_dst)) per edge/head (er broadcast to edges
    via one-hot selD matmuls on the PE); msg = v * z_src (DVE); per-dst-tile
    segment sums via one-hot selT matmuls accumulating in PSUM.  Self-loop
    contribution (v_self, v_self*z_dst) is added from the SBUF-resident dst
    tiles.  Final: (po + v_self*z_dst)/(ps + v_self) + bias.
  No inter-core communication; host concatenates the 8 dst slices.
"""
import os

import numpy as np
import ml_dtypes

import bass_rust
import concourse.bacc as bacc
import concourse.bass as bass
import concourse.mybir as mybir
import concourse.tile as tile

BF = ml_dtypes.bfloat16
F32 = np.float32

NS, ND, E, DIN, H, DH = 10000, 10000, 320000, 512, 8, 64
NEG = 0.2
NCORES = 8
DPC = ND // NCORES            # 1250 dst nodes per core
NDSTT = 10                    # dst tiles per core
SRCPAD = 10112                # src rows padded to 79 tiles of 128
NSRCT = SRCPAD // 128         # 79
NXT = NSRCT + NDSTT           # 89 xT tiles; 0..78 src, 79..88 dst
ROW = 640                     # zel row stride elems (bf16); 520 used
PANEL = 1024                  # phase-1 node panel (4 subtiles of 128)
NXTP = 96                    # xT tiles padded to a PANEL multiple
ZG_BUFS = 2


# ----------------------------------------------------------------- host prep
def _wrap_idx(idx, k):
    """dma_gather index layout: idx i -> [i % 16, i // 16], replicated 8x.
    idx shorter than k is padded with 0 (row 0 gathered, masked by sel)."""
    full = np.zeros(k, np.int64)
    full[:len(idx)] = idx
    w = np.zeros((16, k // 16), np.int16)
    w[np.arange(k) % 16, np.arange(k) // 16] = full
    return np.tile(w, (8, 1))


def _pack_tile(srcs, dsts):
    """Greedy pair edges with consecutive src ids.

    Returns (idx2, dst2a, dst2b, idx1, dst1): R2 descriptors cover rows
    (s, s+1) for edge pairs; R1 descriptors cover single rows."""
    order = np.argsort(srcs, kind="stable")
    s, d = srcs[order], dsts[order]
    n = len(s)
    idx2, d2a, d2b, idx1, d1 = [], [], [], [], []
    i = 0
    while i < n:
        if i + 1 < n and s[i + 1] == s[i] + 1:
            idx2.append(s[i]); d2a.append(d[i]); d2b.append(d[i + 1])
            i += 2
        else:
            idx1.append(s[i]); d1.append(d[i])
            i += 1
    return (np.array(idx2, np.int64), np.array(d2a, np.int64),
            np.array(d2b, np.int64), np.array(idx1, np.int64),
            np.array(d1, np.int64))


def _host_prep(x_src, x_dst, edge_src, edge_dst, W, attn_l, attn_r, bias):
    Al = np.zeros((DIN, H), F32)
    Ar = np.zeros((DIN, H), F32)
    for h in range(H):
        Al[h * DH:(h + 1) * DH, h] = attn_l[h]
        Ar[h * DH:(h + 1) * DH, h] = attn_r[h]
    Wext = np.concatenate([W, W @ Al, W @ Ar], 1).astype(BF)  # [512, 528]
    bias_rep = np.tile(bias[None, :].astype(F32), (128, 1))   # [128, 512]

    edge_src = edge_src.astype(np.int64)
    edge_dst = edge_dst.astype(np.int64)

    packs = [[None] * NDSTT for _ in range(NCORES)]
    k2s = [0] * NDSTT
    k1s = [0] * NDSTT
    for c in range(NCORES):
        d0 = c * DPC
        m = (edge_dst >= d0) & (edge_dst < d0 + DPC)
        es, ed = edge_src[m], edge_dst[m] - d0
        for t in range(NDSTT):
            sel = (ed >= t * 128) & (ed < (t + 1) * 128)
            pk = _pack_tile(es[sel], ed[sel] - t * 128)
            packs[c][t] = pk
            k2s[t] = max(k2s[t], len(pk[0]))
            k1s[t] = max(k1s[t], len(pk[3]))
    k2s = [((k + 127) // 128) * 128 for k in k2s]
    k1s = [((k + 127) // 128) * 128 for k in k1s]
    nchs = [2 * ((k2 + 127) // 128) + (k1 + 127) // 128
            for k2, k1 in zip(k2s, k1s)]
    o2 = np.cumsum([0] + k2s).tolist()     # zidx2 offsets (idx units)
    o1 = np.cumsum([0] + k1s).tolist()
    osel = np.cumsum([0] + [n * 128 for n in nchs]).tolist()

    per_core = []
    for c in range(NCORES):
        zidx2 = np.zeros((128, o2[-1] // 16), np.int16)
        zidx1 = np.zeros((128, o1[-1] // 16), np.int16)
        selT = np.zeros((128, osel[-1]), BF)
        selD = np.zeros((128, osel[-1]), BF)
        for t in range(NDSTT):
            k2, k1, nch = k2s[t], k1s[t], nchs[t]
            c2 = (k2 + 127) // 128
            idx2, d2a, d2b, idx1, d1 = packs[c][t]
            zidx2[:, o2[t] // 16:o2[t + 1] // 16] = _wrap_idx(idx2, k2)
            zidx1[:, o1[t] // 16:o1[t + 1] // 16] = _wrap_idx(idx1, k1)
            # edge -> (partition, chunk) slots
            sm = np.zeros((nch, 128, 128), F32)   # [chunk, edge part, dst]
            j2 = np.arange(len(idx2))
            sm[2 * (j2 // 128), j2 % 128, d2a] = 1.0
            sm[2 * (j2 // 128) + 1, j2 % 128, d2b] = 1.0
            j1 = np.arange(len(idx1))
            sm[2 * c2 + j1 // 128, j1 % 128, d1] = 1.0
            base = osel[t]
            for ch in range(nch):
                selT[:, base + ch * 128:base + (ch + 1) * 128] = \
                    sm[ch].astype(BF)
                selD[:, base + ch * 128:base + (ch + 1) * 128] = \
                    sm[ch].T.astype(BF)

        # xT: cols 0..9999 src nodes, cols 10112.. own dst slice
        xc = np.zeros((NXTP * 128, DIN), F32)
        xc[:NS] = x_src
        xc[SRCPAD:SRCPAD + DPC] = x_dst[c * DPC:(c + 1) * DPC]
        xT = np.ascontiguousarray(xc.T).astype(BF)            # [512, 11776]
        per_core.append(dict(xT=xT, selT=selT, selD=selD,
                             zidx2=zidx2, zidx1=zidx1))

    shared = dict(Wext=Wext, bias_rep=bias_rep)
    return shared, per_core, k2s, k1s


# ------------------------------------------------------------- bass program
def _build_nc(k2s, k1s):
    nchs = [2 * ((k2 + 127) // 128) + (k1 + 127) // 128
            for k2, k1 in zip(k2s, k1s)]
    o2 = np.cumsum([0] + k2s).tolist()
    o1 = np.cumsum([0] + k1s).tolist()
    osel = np.cumsum([0] + [n * 128 for n in nchs]).tolist()
    nch_max = max(nchs)
    nc = bacc.Bacc("TRN2", target_bir_lowering=False, debug=False)
    dt = mybir.dt

    xT_d = nc.dram_tensor("xT", [DIN, NXTP * 128], dt.bfloat16,
                          kind="ExternalInput")
    W_d = nc.dram_tensor("Wext", [DIN, 528], dt.bfloat16, kind="ExternalInput")
    bias_d = nc.dram_tensor("bias_rep", [128, 512], dt.float32,
                            kind="ExternalInput")
    selT_d = nc.dram_tensor("selT", [128, osel[-1]], dt.bfloat16,
                            kind="ExternalInput")
    selD_d = nc.dram_tensor("selD", [128, osel[-1]], dt.bfloat16,
                            kind="ExternalInput")
    zidx2_d = nc.dram_tensor("zidx2", [128, o2[-1] // 16], dt.int16,
                             kind="ExternalInput")
    zidx1_d = nc.dram_tensor("zidx1", [128, o1[-1] // 16], dt.int16,
                             kind="ExternalInput")
    out_d = nc.dram_tensor("out", [NDSTT * 128, 512], dt.float32,
                           kind="ExternalOutput")
    zel_d = nc.dram_tensor("zel_tab", [SRCPAD, ROW], dt.bfloat16)

    with tile.TileContext(nc) as tc:
        with tc.tile_pool(name="const", bufs=1) as cpool:
            wsb = cpool.tile([128, 4 * 528], dt.bfloat16)
            for k in range(4):
                nc.sync.dma_start(wsb[:, k * 528:(k + 1) * 528],
                                  W_d[k * 128:(k + 1) * 128, :])
            bias_sb = cpool.tile([128, 512], dt.float32)
            nc.sync.dma_start(bias_sb[:], bias_d[:])
            zidx2_sb = cpool.tile([128, o2[-1] // 16], dt.int16)
            nc.sync.dma_start(zidx2_sb[:], zidx2_d[:])
            zidx1_sb = cpool.tile([128, o1[-1] // 16], dt.int16)
            nc.sync.dma_start(zidx1_sb[:], zidx1_d[:])
            zdst = cpool.tile([128, NDSTT, 528], dt.bfloat16)
            vself = cpool.tile([128, NDSTT, 8], dt.float32)

            with tc.tile_pool(name="zg", bufs=ZG_BUFS) as zgpool:
                zgt = [zgpool.tile([128, nch_max, ROW], dt.bfloat16,
                                   tag="zg", name=f"zg{i}")
                       for i in range(NDSTT)]

                # ---- phase 1: z/el for src tiles (DRAM zel_tab) + z/el/er
                # for dst tiles (SBUF).  Tiles 0..78 src, 79..88 dst.
                ph1 = tc.tile_pool(name="xp", bufs=2)
                xpool = ph1.__enter__()

                # zel write buffers: full 640-elem rows with the 520:640 tail
                # zeroed once, so zel_tab holds no garbage (gathers read whole
                # rows).
                zel_bufs = [cpool.tile([128, ROW], dt.bfloat16,
                                       name=f"zelbuf{i}") for i in range(3)]
                for zb in zel_bufs:
                    nc.vector.memset(zb[:, 520:ROW], 0)
                ph1c = tc.tile_pool(name="p1", bufs=3, space="PSUM")
                p1pool = ph1c.__enter__()
                ph1d = tc.tile_pool(name="p1b", bufs=3, space="PSUM")
                p1bpool = ph1d.__enter__()
                for p in range(NXTP * 128 // PANEL):
                    xp = xpool.tile([128, 4 * PANEL], dt.bfloat16)
                    for k in range(4):
                        nc.sync.dma_start(
                            xp[:, k * PANEL:(k + 1) * PANEL],
                            xT_d[k * 128:(k + 1) * 128,
                                 p * PANEL:(p + 1) * PANEL])
                    for m in range(PANEL // 128):
                        gt = p * (PANEL // 128) + m   # global tile index
                        if gt >= NXT:
                            continue
                        zps = p1pool.tile([128, 512], dt.float32, space="PSUM")
                        lps = p1bpool.tile([128, 16], dt.float32, space="PSUM")
                        for k in range(4):
                            lhsT = xp[:, k * PANEL + m * 128:
                                      k * PANEL + (m + 1) * 128]
                            nc.tensor.matmul(zps[:], lhsT,
                                             wsb[:, k * 528:k * 528 + 512],
                                             start=(k == 0), stop=(k == 3))
                            nc.tensor.matmul(lps[:], lhsT,
                                             wsb[:, k * 528 + 512:(k + 1) * 528],
                                             start=(k == 0), stop=(k == 3))
                        if gt >= NSRCT:
                            # dst tile: keep z|el|er in SBUF
                            dst_t = gt - NSRCT
                            if gt % 2 == 0:
                                nc.vector.tensor_copy(zdst[:, dst_t, 0:512],
                                                      zps[:])
                                nc.scalar.activation(
                                    zdst[:, dst_t, 512:528], lps[:],
                                    mybir.ActivationFunctionType.Copy)
                            else:
                                nc.scalar.activation(
                                    zdst[:, dst_t, 0:512], zps[:],
                                    mybir.ActivationFunctionType.Copy)
                                nc.vector.tensor_copy(zdst[:, dst_t, 512:528],
                                                      lps[:])
                        else:
                            st = gt                   # src tile index
                            zel_sb = zel_bufs[st % 3]
                            if gt % 2 == 0:
                                nc.vector.tensor_copy(zel_sb[:, 0:512], zps[:])
                                nc.scalar.activation(
                                    zel_sb[:, 512:520], lps[:, 0:8],
                                    mybir.ActivationFunctionType.Copy)
                            else:
                                nc.scalar.activation(
                                    zel_sb[:, 0:512], zps[:],
                                    mybir.ActivationFunctionType.Copy)
                                nc.vector.tensor_copy(zel_sb[:, 512:520],
                                                      lps[:, 0:8])
                            row0 = st * 128
                            nc.sync.dma_start(zel_d[row0:row0 + 128, :],
                                              zel_sb[:])
                ph1d.__exit__(None, None, None)
                ph1c.__exit__(None, None, None)
                ph1.__exit__(None, None, None)

                # vself[d, t, h] = exp(lrelu(el + er)) for the dst self loops
                vtmp = cpool.tile([128, NDSTT, 8], dt.float32)
                nc.vector.tensor_tensor(vtmp[:], zdst[:, :, 512:520],
                                        zdst[:, :, 520:528],
                                        op=mybir.AluOpType.add)
                nc.vector.scalar_tensor_tensor(
                    vtmp[:], vtmp[:], NEG, vtmp[:],
                    op0=mybir.AluOpType.mult, op1=mybir.AluOpType.max)
                nc.scalar.activation(vself[:], vtmp[:],
                                     mybir.ActivationFunctionType.Exp)

                # R2 gathers read 2 consecutive table rows per descriptor:
                # overlapping-window source AP [[640, SRCPAD-1], [1, 1280]]
                zel2_ap = zel_d[:].copy()
                zel2_ap.ap = bass_rust.VecI64Pair(
                    [[ROW, SRCPAD - 1], [1, 2 * ROW]])

                # ---- phase 2: per dst tile gather + attention + aggregation
                with (
                    tc.tile_pool(name="sel", bufs=2) as selpool,
                    tc.tile_pool(name="sc", bufs=3) as scpool,
                    tc.tile_pool(name="vx", bufs=2) as vxpool,
                    tc.tile_pool(name="eo", bufs=2) as eopool,
                    tc.tile_pool(name="p2", bufs=2, space="PSUM") as p2pool,
                    tc.tile_pool(name="p2b", bufs=2, space="PSUM") as p2bpool,
                    tc.tile_pool(name="p2c", bufs=2, space="PSUM") as p2cpool,
                ):
                    def emit_seld_peer(t):
                        # seld load + er-broadcast matmuls for tile t.  Called
                        # one tile ahead so pe_er(t) is not queued behind the
                        # full po/ps chain of tile t-1 on the PE.
                        ncht = nchs[t]
                        seld = selpool.tile([128, nch_max * 128],
                                            dt.bfloat16, tag="seld",
                                            name=f"seld{t}")
                        nc.sync.dma_start(
                            seld[:, 0:ncht * 128],
                            selD_d[:, osel[t]:osel[t + 1]])
                        pe_er = p2cpool.tile([128, nch_max, 8], dt.float32,
                                             space="PSUM", tag="peer", name=f"peer{t}")
                        for ch in range(ncht):
                            nc.tensor.matmul(pe_er[:, ch, :],
                                             seld[:, ch * 128:(ch + 1) * 128],
                                             zdst[:, t, 520:528],
                                             start=True, stop=True,
                                             skip_group_check=True)
                        return pe_er

                    pe_ers = {0: emit_seld_peer(0)}
                    for t in range(NDSTT):
                        k2, k1, nch = k2s[t], k1s[t], nchs[t]
                        c2 = (k2 + 127) // 128
                        last_tile = t == NDSTT - 1
                        sel = selpool.tile([128, nch_max * 128], dt.bfloat16,
                                           tag="sel")
                        nc.sync.dma_start(
                            sel[:, 0:nch * 128],
                            selT_d[:, osel[t]:osel[t + 1]])

                        zg = zgt[t]

                        def gather_r2():
                            zg2 = zg[:, 0:2 * c2, :].rearrange(
                                "p (a b) c -> p a (b c)", b=2)
                            nc.gpsimd.dma_gather(
                                zg2, zel2_ap,
                                zidx2_sb[:, o2[t] // 16:o2[t + 1] // 16],
                                num_idxs=k2, num_idxs_reg=k2,
                                elem_size=2 * ROW, elem_step=ROW,
                                single_packet=False)

                        def gather_r1():
                            nc.gpsimd.dma_gather(
                                zg[:, 2 * c2:nch, :], zel_d[:],
                                zidx1_sb[:, o1[t] // 16:o1[t + 1] // 16],
                                num_idxs=k1, num_idxs_reg=k1, elem_size=ROW,
                                single_packet=False)

                        # For the last tile, gather R1 first so the final
                        # post-gather chain is the short R2 pass.
                        regions = ((0, 2 * c2), (2 * c2, nch))
                        if last_tile:
                            gather_r1()
                            gather_r2()
                            regions = ((2 * c2, nch), (0, 2 * c2))
                        else:
                            gather_r2()
                            gather_r1()

                        pe_er = pe_ers.pop(t)
                        first_ch = regions[0][0]
                        last_ch = regions[-1][1] - 1

                        lt = scpool.tile([128, nch_max, 8], dt.float32,
                                         tag="lt")
                        vb = scpool.tile([128, nch_max, 8], dt.bfloat16,
                                         tag="vb")
                        po = p2pool.tile([128, 512], dt.float32, space="PSUM")
                        ps = p2bpool.tile([128, 8], dt.float32, space="PSUM")
                        GRP = 8
                        for ri, (r0, r1) in enumerate(regions):
                            nc.vector.tensor_tensor(
                                lt[:, r0:r1, :], zg[:, r0:r1, 512:520],
                                pe_er[:, r0:r1, :], op=mybir.AluOpType.add)
                            nc.vector.scalar_tensor_tensor(
                                lt[:, r0:r1, :], lt[:, r0:r1, :], NEG,
                                lt[:, r0:r1, :],
                                op0=mybir.AluOpType.mult,
                                op1=mybir.AluOpType.max)
                            nc.scalar.activation(
                                vb[:, r0:r1, :], lt[:, r0:r1, :],
                                mybir.ActivationFunctionType.Exp)
                            # msg = v * z (in place) interleaved with the
                            # segment sums per chunk group.  v is expanded to
                            # full 512 width on the scalar engine so the DVE
                            # multiply runs both operands step-1 (2x bf16).
                            for g0 in range(r0, r1, GRP):
                                g1 = min(g0 + GRP, r1)
                                ng = g1 - g0
                                vbig = vxpool.tile([128, GRP, 512],
                                                   dt.bfloat16, tag="vbig")
                                v4 = vbig[:, 0:ng, :].rearrange(
                                    "p c (h d) -> p c h d", d=DH)
                                nc.scalar.activation(
                                    v4,
                                    vb[:, g0:g1, :].to_broadcast(
                                        [128, ng, 8, DH]),
                                    mybir.ActivationFunctionType.Copy)
                                nc.vector.tensor_tensor(
                                    zg[:, g0:g1, 0:512],
                                    zg[:, g0:g1, 0:512],
                                    vbig[:, 0:ng, :],
                                    op=mybir.AluOpType.mult)
                                for ch in range(g0, g1):
                                    sl = sel[:, ch * 128:(ch + 1) * 128]
                                    nc.tensor.matmul(po[:], sl,
                                                     zg[:, ch, 0:512],
                                                     start=(ch == first_ch),
                                                     stop=(ch == last_ch))
                                    nc.tensor.matmul(ps[:], sl, vb[:, ch, :],
                                                     start=(ch == first_ch),
                                                     stop=(ch == last_ch))
                            # hoist next tile's seld load + er matmuls ahead
                            # of this tile's second region pass on the PE
                            if ri == 0 and t + 1 < NDSTT:
                                pe_ers[t + 1] = emit_seld_peer(t + 1)

                        # out = (po + vself*z_dst) / (ps + vself) + bias
                        ssb = scpool.tile([128, 8], dt.float32, tag="ssb")
                        nc.vector.tensor_tensor(ssb[:], ps[:], vself[:, t, :],
                                                op=mybir.AluOpType.add)
                        nc.vector.reciprocal(ssb[:], ssb[:])
                        msf = scpool.tile([128, 512], dt.float32, tag="msf")
                        m4 = msf[:].rearrange("p (h d) -> p h d", d=DH)
                        nc.vector.tensor_tensor(
                            m4, zdst[:, t, 0:512].rearrange(
                                "p (h d) -> p h d", d=DH),
                            vself[:, t, :].to_broadcast([128, 8, DH]),
                            op=mybir.AluOpType.mult)
                        osb = eopool.tile([128, 512], dt.float32)
                        nc.vector.tensor_tensor(osb[:], po[:], msf[:],
                                                op=mybir.AluOpType.add)
                        o4 = osb[:].rearrange("p (h d) -> p h d", d=DH)
                        nc.vector.tensor_tensor(
                            o4, o4, ssb[:].to_broadcast([128, 8, DH]),
                            op=mybir.AluOpType.mult)
                        nc.vector.tensor_tensor(osb[:], osb[:], bias_sb[:],
                                                op=mybir.AluOpType.add)
                        nc.sync.dma_start(out_d[t * 128:(t + 1) * 128, :],
                                          osb[:])
    nc.compile()
    return nc


# ------------------------------------------------------------------- driver
def kernel(x_src, x_dst, edge_src, edge_dst, W, attn_l, attn_r, bias):
    shared, per_core, k2s, k1s = _host_prep(
        np.asarray(x_src), np.asarray(x_dst), np.asarray(edge_src),
        np.asarray(edge_dst), np.asarray(W), np.asarray(attn_l),
        np.asarray(attn_r), np.asarray(bias))

    nc = _build_nc(k2s, k1s)

    in_maps = []
    for c in range(NCORES):
        in_maps.append({"xT": per_core[c]["xT"], "Wext": shared["Wext"],
                        "bias_rep": shared["bias_rep"],
                        "selT": per_core[c]["selT"],
                        "selD": per_core[c]["selD"],
                        "zidx2": per_core[c]["zidx2"],
                        "zidx1": per_core[c]["zidx1"]})

    if os.environ.get("KERNEL_SIM"):
        from concourse.bass_interp import CoreSim
        sim = CoreSim(nc, trace=False)
        for name, arr in in_maps[int(os.environ.get("KERNEL_SIM_CORE", "0"))].items():
            sim.tensor(name)[:] = arr
        sim.simulate()
        out = np.array(sim.tensor("out"))
        return np.concatenate([out[:DPC]] * NCORES, 0)  # core-0 slice only

    from concourse.bass_utils import run_bass_kernel_spmd
    res = run_bass_kernel_spmd(nc, in_maps, core_ids=list(range(NCORES)),
                               trace=bool(os.environ.get("KERNEL_TRACE")))
    global LAST_RESULTS
    LAST_RESULTS = res
    return np.concatenate([r["out"][:DPC] for r in res.results], 0)


LAST_RESULTS = None
